# revision 7
# baseline (speedup 1.0000x reference)
"""DiffMamba cross-attention kernel for 8 Trainium2 NeuronCores.

Problem (hardcoded shapes): B=4, SQ=SK=2048, D=1024, H=16, HD=64.
  q = x @ Wq.T ; k = e @ Wk.T ; v = e @ Wv.T      (per-head split, HD=64)
  out = softmax(q k^T / 8) v                       (merged heads)

Sharding: core c -> (batch b = c//2, head-group hg = c%2).  Each core owns
one batch element and 8 of the 16 heads (rows hg*512:(hg+1)*512 of W), so
all cores are fully independent (no collectives).

Host pre-transposes everything so the device kernel is transpose-free:
  xT [1024,2048], eT [1024,2048], wqT/wkT/wvT [1024,512]  (wqT pre-scaled 1/8)
Device computes outT [512,2048] = (attention output).T; host transposes back.

v2 (this file): single fused pipeline.  The run is ScalarE-bound (exp of
33.5M score elements/core at 1 elem/lane/cycle @1.2GHz + ~350cyc/op fixed
overhead = ~295us of ACT with [128,1024] ops, which PSUM sizing pins).  The
v1 kernel ran projections as a serial ~95us phase before attention; here the
projection matmuls are emitted as *filler units* inside the attention
j-loop, placed exactly at the PE queue's stall point (between the score
matmuls and the ctx matmuls, which wait on the exp).  That (a) hides the
whole projection phase under the ACT stream, and (b) keeps the PE
continuously busy so it holds its high p-state (it measured ~1.33GHz in v1
because it idled every iteration and never ramped).

Per j-slot steady state (target):  PE: score pair (concurrent on rows 0-63 /
64-127 via tile_position) + 2 filler matmuls + 2 ctx matmuls ~= 1.07us at
2.4GHz; ACT: one [128,1024] exp = 1.15us.  ACT is the roofline.

PSUM budget (16KB/partition): score tiles 2x[128,1024]f32 (4 banks) +
ctx 3x[65,512]f32 (3 banks, a/b for current chunk + 1 for overlap) +
projection accumulator [128,512]f32 (1 bank).
"""

import os
import sys
from collections import deque

import numpy as np

_REPO = "/opt/trn_rl_repo"
if os.path.isdir(_REPO) and _REPO not in sys.path:
    sys.path.insert(0, _REPO)

import concourse.bass as bass
import concourse.tile as tile
from concourse import bacc
from concourse import mybir
from concourse.bass_utils import run_bass_kernel_spmd

F32 = mybir.dt.float32
BF16 = mybir.dt.bfloat16
PSUM = bass.MemorySpace.PSUM
EXP = mybir.ActivationFunctionType.Exp

B, S, D = 4, 2048, 1024
DL = 512          # head dims per core (8 heads x 64)
HL = 8            # local heads
NP = 4            # local head pairs
KT = D // 128     # 8 contraction tiles
NCORES = 8

_CACHE = {}
LAST_RESULT = None  # BassKernelResults of the most recent run (for profiling)


def _build_program():
    # Bacc (not raw Bass): its compile pipeline splits multi-sem waits into
    # EventSemaphore instructions and moves matmul waits onto ldweights --
    # walrus rejects >1 sync wait on most instructions.
    nc = bacc.Bacc()
    xT_h = nc.declare_dram_parameter("xT", [D, S], BF16, isOutput=False)
    eT_h = nc.declare_dram_parameter("eT", [D, S], BF16, isOutput=False)
    wqT_h = nc.declare_dram_parameter("wqT", [D, DL], BF16, isOutput=False)
    wkT_h = nc.declare_dram_parameter("wkT", [D, DL], BF16, isOutput=False)
    wvT_h = nc.declare_dram_parameter("wvT", [D, DL], BF16, isOutput=False)
    outT_h = nc.declare_dram_parameter("outT", [DL, S], F32, isOutput=True)

    # [D, N] viewed as [128, KT, N]: partition p, ktile k -> row k*128+p
    xT_v = xT_h[:].rearrange("(k p) n -> p k n", p=128)
    eT_v = eT_h[:].rearrange("(k p) n -> p k n", p=128)
    wqT_v = wqT_h[:].rearrange("(k p) n -> p k n", p=128)
    wkT_v = wkT_h[:].rearrange("(k p) n -> p k n", p=128)
    wvT_v = wvT_h[:].rearrange("(k p) n -> p k n", p=128)

    with tile.TileContext(nc) as tc:
        with (
            tc.tile_pool(name="persist", bufs=1) as persist,
            tc.tile_pool(name="psproj", bufs=1, space=PSUM) as psp,
            tc.tile_pool(name="stp", bufs=2, space=PSUM) as stp,
            tc.tile_pool(name="ctxp", bufs=3, space=PSUM) as ctxp,
            tc.tile_pool(name="ptp", bufs=3) as ptp,
            tc.tile_pool(name="stg", bufs=2) as stgp,
            tc.tile_pool(name="dnp", bufs=2) as dnp,
        ):
            qT = persist.tile([128, NP, S], BF16, tag="qT")
            kT = persist.tile([128, NP, S], BF16, tag="kT")
            # v augmented: per SK tile, per head: 64 v-dims + ones column
            vA = persist.tile([128, 16, HL, 65], BF16, tag="vA")
            zbias = persist.tile([128, 1], F32, tag="zbias")
            wq = persist.tile([128, KT, DL], BF16, tag="wq")
            wk = persist.tile([128, KT, DL], BF16, tag="wk")
            wv = persist.tile([128, KT, DL], BF16, tag="wv")
            # full activations stay resident so projections can be emitted
            # at any point of the pipeline (32KB/partition each)
            eT = persist.tile([128, KT, S], BF16, tag="eT")
            xT = persist.tile([128, KT, S], BF16, tag="xT")

            nc.vector.memset(zbias[:], 0.0)
            nc.vector.memset(vA[:, :, :, 64:65], 1.0)

            # ---- input DMAs: K-path first (kT/vA gate the pipeline) ----
            nc.sync.dma_start(wk[:], wkT_v)
            for n in range(4):
                nsl = slice(n * 512, (n + 1) * 512)
                nc.sync.dma_start(eT[:, :, nsl], eT_v[:, :, nsl])
                if n == 0:
                    nc.sync.dma_start(wv[:], wvT_v)
                if n == 1:
                    nc.sync.dma_start(wq[:], wqT_v)
                    nc.sync.dma_start(xT[:, :, 0:512], xT_v[:, :, 0:512])
            for n in range(1, 4):
                nsl = slice(n * 512, (n + 1) * 512)
                nc.sync.dma_start(xT[:, :, nsl], xT_v[:, :, nsl])

            # ---- projection unit generators -------------------------------
            # A "group" is one PSUM accumulation (8 k-step matmuls + one
            # PSUM->SBUF copy), split into single-matmul closures so filler
            # placement is fine-grained.  All groups share the 1-bank psp
            # pool; group N+1's first matmul auto-waits group N's copy.

            def proj_group(stat_fn, mov_fn, width, write_fn):
                st = {}

                def mk_mm(k):
                    def f():
                        if k == 0:
                            st["ps"] = psp.tile(
                                [128, width], F32, name="psproj", tag="psproj"
                            )
                        nc.tensor.matmul(
                            st["ps"][:],
                            stat_fn(k),
                            mov_fn(k),
                            start=(k == 0),
                            stop=(k == KT - 1),
                        )

                    return f

                def mk_copy():
                    def f():
                        write_fn(st["ps"])

                    return f

                return [mk_mm(k) for k in range(KT)] + [mk_copy()]

            def kt_units(p, n):
                psl = slice(p * 128, (p + 1) * 128)
                nsl = slice(n * 512, (n + 1) * 512)
                return proj_group(
                    lambda k: wk[:, k, psl],
                    lambda k: eT[:, k, nsl],
                    512,
                    lambda ps: nc.vector.tensor_copy(kT[:, p, nsl], ps[:]),
                )

            def qt_units(p, c):
                psl = slice(p * 128, (p + 1) * 128)
                csl = slice(c * 512, (c + 1) * 512)
                return proj_group(
                    lambda k: wq[:, k, psl],
                    lambda k: xT[:, k, csl],
                    512,
                    lambda ps: nc.vector.tensor_copy(qT[:, p, csl], ps[:]),
                )

            def va_units(g, mj):
                # pair-group g in {0,1}: heads 4g..4g+3, wv cols 256g..+256
                vsl = slice(g * 256, (g + 1) * 256)
                msl = slice(mj * 128, (mj + 1) * 128)
                return proj_group(
                    lambda k: eT[:, k, msl],
                    lambda k: wv[:, k, vsl],
                    256,
                    lambda ps: nc.vector.tensor_copy(
                        vA[:, mj, 4 * g : 4 * g + 4, 0:64],
                        ps[:].rearrange("p (h d) -> p h d", h=4),
                    ),
                )

            # ---- prologue: everything attention(pair 0, chunk 0) needs ----
            for n in range(4):
                for u in kt_units(0, n):
                    u()
                for mj in range(4 * n, 4 * n + 4):
                    for u in va_units(0, mj):
                        u()
            for u in qt_units(0, 0):
                u()

            # ---- filler queue: the rest of the projections, deadline-ordered.
            # A unit POPPED LATE doesn't stall -- it CORRUPTS: a read emitted
            # before its producing write binds to stale SBUF (program-order
            # dep tracking).  So each attn(p,c) force-drains its prefix via
            # the `need` markers; the in-loop rate pops just keep it smooth.
            filler = []
            need = {}
            for c in (1, 2, 3):
                filler.extend(qt_units(0, c))
                need[(0, c)] = len(filler)
            for n in range(4):
                filler.extend(kt_units(1, n))
            filler.extend(qt_units(1, 0))
            need[(1, 0)] = len(filler)
            for c in (1, 2, 3):
                filler.extend(qt_units(1, c))
                need[(1, c)] = len(filler)
            for mj in range(16):
                filler.extend(va_units(1, mj))
            for n in range(4):
                filler.extend(kt_units(2, n))
            filler.extend(qt_units(2, 0))
            need[(2, 0)] = len(filler)
            for c in (1, 2, 3):
                filler.extend(qt_units(2, c))
                need[(2, c)] = len(filler)
            for n in range(4):
                filler.extend(kt_units(3, n))
            filler.extend(qt_units(3, 0))
            need[(3, 0)] = len(filler)
            for c in (1, 2, 3):
                filler.extend(qt_units(3, c))
                need[(3, c)] = len(filler)

            fpos = [0]

            def pop_filler(k):
                stop = min(fpos[0] + k, len(filler))
                while fpos[0] < stop:
                    filler[fpos[0]]()
                    fpos[0] += 1

            def drain_to(marker):
                while fpos[0] < marker:
                    filler[fpos[0]]()
                    fpos[0] += 1

            # ---- attention ------------------------------------------------
            rate = {0: 2, 1: 3, 2: 2, 3: 2}
            for p in range(NP):
                stage_a = stgp.tile([64, S], F32, tag="stage_a")
                stage_b = stgp.tile([64, S], F32, tag="stage_b")
                for c in range(4):
                    drain_to(need.get((p, c), 0))
                    csl = slice(c * 512, (c + 1) * 512)
                    ctx_a = ctxp.tile([65, 512], F32, tag="ctx")
                    ctx_b = ctxp.tile([65, 512], F32, tag="ctx")
                    for j in range(16):
                        jsl = slice(j * 128, (j + 1) * 128)
                        st = stp.tile([128, 1024], F32, tag="st")
                        nc.tensor.matmul(
                            st[:, 0:512],
                            kT[0:64, p, jsl],
                            qT[0:64, p, csl],
                            start=True,
                            stop=True,
                        )
                        nc.tensor.matmul(
                            st[:, 512:1024],
                            kT[64:128, p, jsl],
                            qT[64:128, p, csl],
                            start=True,
                            stop=True,
                        )
                        # filler lands where the PE queue would stall
                        # (ctx waits on the exp)
                        pop_filler(rate[p])
                        pt = ptp.tile([128, 1024], BF16, tag="pt")
                        nc.scalar.activation(pt[:], st[:], EXP, bias=zbias[:, 0:1])
                        nc.tensor.matmul(
                            ctx_a[:],
                            vA[:, j, 2 * p, :],
                            pt[:, 0:512],
                            start=(j == 0),
                            stop=(j == 15),
                        )
                        nc.tensor.matmul(
                            ctx_b[:],
                            vA[:, j, 2 * p + 1, :],
                            pt[:, 512:1024],
                            start=(j == 0),
                            stop=(j == 15),
                        )
                    # normalization: denominators live in ctx row 64; the
                    # whole chain runs on DVE/GPSIMD so the PE/ACT streams
                    # never wait on it
                    pop_filler(3)
                    dn = dnp.tile([65, 1024], F32, tag="dn")
                    nc.vector.tensor_copy(dn[64:65, 0:512], ctx_a[64:65, :])
                    nc.vector.tensor_copy(dn[64:65, 512:1024], ctx_b[64:65, :])
                    # reshape the 1024 denominators across 32 partitions so
                    # the bit-exact reciprocal runs 32 lanes wide
                    dnR = dnp.tile([32, 32], F32, tag="dnR")
                    nc.sync.dma_start(dnR[:], dn[64:65, :])
                    rcR = dnp.tile([32, 32], F32, tag="rcR")
                    nc.vector.reciprocal(rcR[:], dnR[:])
                    rc0 = dnp.tile([1, 1024], F32, tag="rc0")
                    nc.sync.dma_start(rc0[:], rcR[:])
                    bcs_a = dnp.tile([64, 512], F32, tag="bcs_a")
                    bcs_b = dnp.tile([64, 512], F32, tag="bcs_b")
                    nc.gpsimd.partition_broadcast(bcs_a[:], rc0[0:1, 0:512])
                    nc.gpsimd.partition_broadcast(bcs_b[:], rc0[0:1, 512:1024])
                    nc.vector.tensor_mul(stage_a[:, csl], ctx_a[0:64, :], bcs_a[:])
                    nc.vector.tensor_mul(stage_b[:, csl], ctx_b[0:64, :], bcs_b[:])
                    # per-chunk output DMA on the idle GPSIMD (SWDGE)
                    nc.gpsimd.dma_start(
                        outT_h[p * 128 : p * 128 + 64, csl],
                        stage_a[:, csl],
                    )
                    nc.gpsimd.dma_start(
                        outT_h[p * 128 + 64 : (p + 1) * 128, csl],
                        stage_b[:, csl],
                    )

            # anything left (shouldn't be): flush
            drain_to(len(filler))

    nc.finalize()
    return nc


def kernel(hidden_states, encoder_hidden_states, Wq, Wk, Wv):
    global LAST_RESULT
    hidden_states = np.asarray(hidden_states, dtype=np.float32)
    encoder_hidden_states = np.asarray(encoder_hidden_states, dtype=np.float32)
    Wq = np.asarray(Wq, dtype=np.float32)
    Wk = np.asarray(Wk, dtype=np.float32)
    Wv = np.asarray(Wv, dtype=np.float32)

    if "nc" not in _CACHE:
        _CACHE["nc"] = _build_program()
    nc = _CACHE["nc"]

    import ml_dtypes

    bf16 = ml_dtypes.bfloat16
    in_maps = []
    for c in range(NCORES):
        b, hg = divmod(c, 2)
        rsl = slice(hg * DL, (hg + 1) * DL)
        in_maps.append(
            {
                "xT": np.ascontiguousarray(hidden_states[b].T).astype(bf16),
                "eT": np.ascontiguousarray(encoder_hidden_states[b].T).astype(bf16),
                # fold the 1/sqrt(HD)=1/8 score scale into Wq
                "wqT": np.ascontiguousarray((Wq[rsl] * 0.125).T).astype(bf16),
                "wkT": np.ascontiguousarray(Wk[rsl].T).astype(bf16),
                "wvT": np.ascontiguousarray(Wv[rsl].T).astype(bf16),
            }
        )

    res = run_bass_kernel_spmd(nc, in_maps, list(range(NCORES)))
    LAST_RESULT = res

    out = np.empty((B, S, D), dtype=np.float32)
    for c in range(NCORES):
        b, hg = divmod(c, 2)
        out[b, :, hg * DL : (hg + 1) * DL] = res.results[c]["outT"].T
    return out


# revision 12
# speedup vs baseline: 1.2871x; 1.2871x over previous
"""DiffMamba cross-attention kernel for 8 Trainium2 NeuronCores.

Problem (hardcoded shapes): B=4, SQ=SK=2048, D=1024, H=16, HD=64.
  q = x @ Wq.T ; k = e @ Wk.T ; v = e @ Wv.T      (per-head split, HD=64)
  out = softmax(q k^T / 8) v                       (merged heads)

Sharding: core c -> (batch b = c//2, head-group hg = c%2).  Each core owns
one batch element and 8 of the 16 heads (rows hg*512:(hg+1)*512 of W), so
all cores are fully independent (no collectives).

Host pre-transposes everything so the device kernel is transpose-free:
  xT [1024,2048], eT [1024,2048], wqT/wkT/wvT [1024,512]  (wqT pre-scaled 1/8)
Device computes outT [512,2048] = (attention output).T; host transposes back.

v2 (this file): single fused pipeline.  The run is ScalarE-bound (exp of
33.5M score elements/core at 1 elem/lane/cycle @1.2GHz + ~350cyc/op fixed
overhead = ~295us of ACT with [128,1024] ops, which PSUM sizing pins).  The
v1 kernel ran projections as a serial ~95us phase before attention; here the
projection matmuls are emitted as *filler units* inside the attention
j-loop, placed exactly at the PE queue's stall point (between the score
matmuls and the ctx matmuls, which wait on the exp).  That (a) hides the
whole projection phase under the ACT stream, and (b) keeps the PE
continuously busy so it holds its high p-state (it measured ~1.33GHz in v1
because it idled every iteration and never ramped).

Per j-slot steady state (target):  PE: score pair (concurrent on rows 0-63 /
64-127 via tile_position) + 2 filler matmuls + 2 ctx matmuls ~= 1.07us at
2.4GHz; ACT: one [128,1024] exp = 1.15us.  ACT is the roofline.

PSUM budget (16KB/partition): score tiles 2x[128,1024]f32 (4 banks) +
ctx 3x[65,512]f32 (3 banks, a/b for current chunk + 1 for overlap) +
projection accumulator [128,512]f32 (1 bank).
"""

import os
import sys
from collections import deque

import numpy as np

_REPO = "/opt/trn_rl_repo"
if os.path.isdir(_REPO) and _REPO not in sys.path:
    sys.path.insert(0, _REPO)

import concourse.bass as bass
import concourse.tile as tile
from concourse import bacc
from concourse import mybir
from concourse.bass_utils import run_bass_kernel_spmd

F32 = mybir.dt.float32
BF16 = mybir.dt.bfloat16
PSUM = bass.MemorySpace.PSUM
EXP = mybir.ActivationFunctionType.Exp

B, S, D = 4, 2048, 1024
DL = 512          # head dims per core (8 heads x 64)
HL = 8            # local heads
NP = 4            # local head pairs
KT = D // 128     # 8 contraction tiles
NCORES = 8

_CACHE = {}
LAST_RESULT = None  # BassKernelResults of the most recent run (for profiling)


def _build_program():
    # Bacc (not raw Bass): its compile pipeline splits multi-sem waits into
    # EventSemaphore instructions and moves matmul waits onto ldweights --
    # walrus rejects >1 sync wait on most instructions.
    nc = bacc.Bacc()
    xT_h = nc.declare_dram_parameter("xT", [D, S], BF16, isOutput=False)
    eT_h = nc.declare_dram_parameter("eT", [D, S], BF16, isOutput=False)
    wqT_h = nc.declare_dram_parameter("wqT", [D, DL], BF16, isOutput=False)
    wkT_h = nc.declare_dram_parameter("wkT", [D, DL], BF16, isOutput=False)
    wvT_h = nc.declare_dram_parameter("wvT", [D, DL], BF16, isOutput=False)
    outT_h = nc.declare_dram_parameter("outT", [DL, S], F32, isOutput=True)

    # [D, N] viewed as [128, KT, N]: partition p, ktile k -> row k*128+p
    xT_v = xT_h[:].rearrange("(k p) n -> p k n", p=128)
    eT_v = eT_h[:].rearrange("(k p) n -> p k n", p=128)
    wqT_v = wqT_h[:].rearrange("(k p) n -> p k n", p=128)
    wkT_v = wkT_h[:].rearrange("(k p) n -> p k n", p=128)
    wvT_v = wvT_h[:].rearrange("(k p) n -> p k n", p=128)

    with tile.TileContext(nc) as tc:
        with (
            tc.tile_pool(name="persist", bufs=1) as persist,
            tc.tile_pool(name="psproj", bufs=2, space=PSUM) as psp,
            tc.tile_pool(name="stp", bufs=2, space=PSUM) as stp,
            tc.tile_pool(name="ctxp", bufs=2, space=PSUM) as ctxp,
            tc.tile_pool(name="ptp", bufs=3) as ptp,
            tc.tile_pool(name="stg", bufs=2) as stgp,
            tc.tile_pool(name="cpyp", bufs=2) as cpyp,
            tc.tile_pool(name="dnp", bufs=2) as dnp,
        ):
            qT = persist.tile([128, NP, S], BF16, tag="qT")
            kT = persist.tile([128, NP, S], BF16, tag="kT")
            # v augmented: per SK tile, per head: 64 v-dims + ones column
            vA = persist.tile([128, 16, HL, 65], BF16, tag="vA")
            zbias = persist.tile([128, 1], F32, tag="zbias")
            wq = persist.tile([128, KT, DL], BF16, tag="wq")
            wk = persist.tile([128, KT, DL], BF16, tag="wk")
            wv = persist.tile([128, KT, DL], BF16, tag="wv")
            # full activations stay resident so projections can be emitted
            # at any point of the pipeline (32KB/partition each)
            eT = persist.tile([128, KT, S], BF16, tag="eT")
            xT = persist.tile([128, KT, S], BF16, tag="xT")

            nc.vector.memset(zbias[:], 0.0)
            nc.vector.memset(vA[:, :, :, 64:65], 1.0)

            # ---- input DMAs: K-path first (kT/vA gate the pipeline) ----
            nc.sync.dma_start(wk[:], wkT_v)
            for n in range(4):
                nsl = slice(n * 512, (n + 1) * 512)
                nc.sync.dma_start(eT[:, :, nsl], eT_v[:, :, nsl])
                if n == 0:
                    nc.sync.dma_start(wv[:], wvT_v)
                if n == 1:
                    nc.sync.dma_start(wq[:], wqT_v)
                    nc.sync.dma_start(xT[:, :, 0:512], xT_v[:, :, 0:512])
            for n in range(1, 4):
                nsl = slice(n * 512, (n + 1) * 512)
                nc.sync.dma_start(xT[:, :, nsl], xT_v[:, :, nsl])

            # ---- projection unit generators -------------------------------
            # A "group" is one PSUM accumulation (8 k-step matmuls + one
            # PSUM->SBUF copy), split into single-matmul closures so filler
            # placement is fine-grained.  All groups share the 1-bank psp
            # pool; group N+1's first matmul auto-waits group N's copy.

            def proj_group(stat_fn, mov_fn, width, write_fn, pool=None):
                st = {}
                use_pool = pool if pool is not None else psp

                def mk_mm(k):
                    def f():
                        if k == 0:
                            st["ps"] = use_pool.tile(
                                [128, width], F32, name="psproj", tag="psproj"
                            )
                        nc.tensor.matmul(
                            st["ps"][:],
                            stat_fn(k),
                            mov_fn(k),
                            start=(k == 0),
                            stop=(k == KT - 1),
                        )

                    return f

                def mk_copy():
                    def f():
                        write_fn(st["ps"])

                    return f

                return [mk_mm(k) for k in range(KT)] + [mk_copy()]

            def kt_units(p, n, pool=None):
                psl = slice(p * 128, (p + 1) * 128)
                nsl = slice(n * 512, (n + 1) * 512)
                return proj_group(
                    lambda k: wk[:, k, psl],
                    lambda k: eT[:, k, nsl],
                    512,
                    lambda ps: nc.vector.tensor_copy(kT[:, p, nsl], ps[:]),
                    pool=pool,
                )

            def qt_units(p, c, pool=None):
                psl = slice(p * 128, (p + 1) * 128)
                csl = slice(c * 512, (c + 1) * 512)
                return proj_group(
                    lambda k: wq[:, k, psl],
                    lambda k: xT[:, k, csl],
                    512,
                    lambda ps: nc.vector.tensor_copy(qT[:, p, csl], ps[:]),
                    pool=pool,
                )

            def va_units(g, mj, pool=None):
                # pair-group g in {0,1}: heads 4g..4g+3, wv cols 256g..+256
                vsl = slice(g * 256, (g + 1) * 256)
                msl = slice(mj * 128, (mj + 1) * 128)
                return proj_group(
                    lambda k: eT[:, k, msl],
                    lambda k: wv[:, k, vsl],
                    256,
                    lambda ps: nc.vector.tensor_copy(
                        vA[:, mj, 4 * g : 4 * g + 4, 0:64],
                        ps[:].rearrange("p (h d) -> p h d", h=4),
                    ),
                    pool=pool,
                )

            # ---- prologue: everything attention(pair 0, chunk 0) needs.
            # psp has 2 banks, so consecutive accumulation groups alternate
            # banks and each group's matmuls overlap the previous group's
            # PSUM->SBUF copy -- no WAR gap.
            for n in range(4):
                for u in kt_units(0, n):
                    u()
                for mj in range(4 * n, 4 * n + 4):
                    for u in va_units(0, mj):
                        u()
            for u in qt_units(0, 0):
                u()

            # ---- filler queue: the rest of the projections, deadline-ordered.
            # A unit POPPED LATE doesn't stall -- it CORRUPTS: a read emitted
            # before its producing write binds to stale SBUF (program-order
            # dep tracking).  So each attn(p,c) force-drains its prefix via
            # the `need` markers; the in-loop rate pops just keep it smooth.
            filler = []
            need = {}
            for c in (1, 2, 3):
                filler.extend(qt_units(0, c))
                need[(0, c)] = len(filler)
            for n in range(4):
                filler.extend(kt_units(1, n))
            filler.extend(qt_units(1, 0))
            need[(1, 0)] = len(filler)
            for c in (1, 2, 3):
                filler.extend(qt_units(1, c))
                need[(1, c)] = len(filler)
            for mj in range(16):
                filler.extend(va_units(1, mj))
            for n in range(4):
                filler.extend(kt_units(2, n))
            filler.extend(qt_units(2, 0))
            need[(2, 0)] = len(filler)
            for c in (1, 2, 3):
                filler.extend(qt_units(2, c))
                need[(2, c)] = len(filler)
            for n in range(4):
                filler.extend(kt_units(3, n))
            filler.extend(qt_units(3, 0))
            need[(3, 0)] = len(filler)
            for c in (1, 2, 3):
                filler.extend(qt_units(3, c))
                need[(3, c)] = len(filler)

            fpos = [0]

            def pop_filler(k):
                stop = min(fpos[0] + k, len(filler))
                while fpos[0] < stop:
                    filler[fpos[0]]()
                    fpos[0] += 1

            def drain_to(marker):
                while fpos[0] < marker:
                    filler[fpos[0]]()
                    fpos[0] += 1

            # ---- attention ------------------------------------------------
            rate = {0: 2, 1: 3, 2: 2, 3: 2}
            for p in range(NP):
                stage_a = stgp.tile([64, S], F32, tag="stage_a")
                stage_b = stgp.tile([64, S], F32, tag="stage_b")
                for c in range(4):
                    drain_to(need.get((p, c), 0))
                    csl = slice(c * 512, (c + 1) * 512)
                    ctx_a = ctxp.tile([65, 512], F32, tag="ctx")
                    ctx_b = ctxp.tile([65, 512], F32, tag="ctx")
                    for j in range(16):
                        jsl = slice(j * 128, (j + 1) * 128)
                        st = stp.tile([128, 1024], F32, tag="st")
                        nc.tensor.matmul(
                            st[:, 0:512],
                            kT[0:64, p, jsl],
                            qT[0:64, p, csl],
                            start=True,
                            stop=True,
                        )
                        nc.tensor.matmul(
                            st[:, 512:1024],
                            kT[64:128, p, jsl],
                            qT[64:128, p, csl],
                            start=True,
                            stop=True,
                        )
                        # filler lands where the PE queue would stall
                        # (ctx waits on the exp)
                        pop_filler(rate[p])
                        pt = ptp.tile([128, 1024], BF16, tag="pt")
                        nc.scalar.activation(pt[:], st[:], EXP, bias=zbias[:, 0:1])
                        nc.tensor.matmul(
                            ctx_a[:],
                            vA[:, j, 2 * p, :],
                            pt[:, 0:512],
                            start=(j == 0),
                            stop=(j == 15),
                        )
                        nc.tensor.matmul(
                            ctx_b[:],
                            vA[:, j, 2 * p + 1, :],
                            pt[:, 512:1024],
                            start=(j == 0),
                            stop=(j == 15),
                        )
                    # normalization: denominators live in ctx row 64.  First
                    # copy both ctx tiles to SBUF so the 2 ctx PSUM banks are
                    # released within ~1us (the next chunk's ctx matmuls
                    # reuse them); then the whole chain runs on DVE/GPSIMD
                    # off the PSUM tiles so PE/ACT never wait on it.
                    pop_filler(3)
                    cpy = cpyp.tile([65, 1024], F32, tag="cpy")
                    nc.vector.tensor_copy(cpy[:, 0:512], ctx_a[:])
                    nc.vector.tensor_copy(cpy[:, 512:1024], ctx_b[:])
                    # reshape the 1024 denominators across 32 partitions so
                    # the bit-exact reciprocal runs 32 lanes wide
                    dnR = dnp.tile([32, 32], F32, tag="dnR")
                    nc.sync.dma_start(dnR[:], cpy[64:65, :])
                    rcR = dnp.tile([32, 32], F32, tag="rcR")
                    nc.vector.reciprocal(rcR[:], dnR[:])
                    rc0 = dnp.tile([1, 1024], F32, tag="rc0")
                    nc.sync.dma_start(rc0[:], rcR[:])
                    bcs_a = dnp.tile([64, 512], F32, tag="bcs_a")
                    bcs_b = dnp.tile([64, 512], F32, tag="bcs_b")
                    nc.gpsimd.partition_broadcast(bcs_a[:], rc0[0:1, 0:512])
                    nc.gpsimd.partition_broadcast(bcs_b[:], rc0[0:1, 512:1024])
                    nc.vector.tensor_mul(stage_a[:, csl], cpy[0:64, 0:512], bcs_a[:])
                    nc.vector.tensor_mul(stage_b[:, csl], cpy[0:64, 512:1024], bcs_b[:])
                    # per-chunk output DMA on the idle GPSIMD (SWDGE)
                    nc.gpsimd.dma_start(
                        outT_h[p * 128 : p * 128 + 64, csl],
                        stage_a[:, csl],
                    )
                    nc.gpsimd.dma_start(
                        outT_h[p * 128 + 64 : (p + 1) * 128, csl],
                        stage_b[:, csl],
                    )

            # anything left (shouldn't be): flush
            drain_to(len(filler))

    nc.finalize()
    return nc


def kernel(hidden_states, encoder_hidden_states, Wq, Wk, Wv):
    global LAST_RESULT
    hidden_states = np.asarray(hidden_states, dtype=np.float32)
    encoder_hidden_states = np.asarray(encoder_hidden_states, dtype=np.float32)
    Wq = np.asarray(Wq, dtype=np.float32)
    Wk = np.asarray(Wk, dtype=np.float32)
    Wv = np.asarray(Wv, dtype=np.float32)

    if "nc" not in _CACHE:
        _CACHE["nc"] = _build_program()
    nc = _CACHE["nc"]

    import ml_dtypes

    bf16 = ml_dtypes.bfloat16
    in_maps = []
    for c in range(NCORES):
        b, hg = divmod(c, 2)
        rsl = slice(hg * DL, (hg + 1) * DL)
        in_maps.append(
            {
                "xT": np.ascontiguousarray(hidden_states[b].T).astype(bf16),
                "eT": np.ascontiguousarray(encoder_hidden_states[b].T).astype(bf16),
                # fold the 1/sqrt(HD)=1/8 score scale into Wq
                "wqT": np.ascontiguousarray((Wq[rsl] * 0.125).T).astype(bf16),
                "wkT": np.ascontiguousarray(Wk[rsl].T).astype(bf16),
                "wvT": np.ascontiguousarray(Wv[rsl].T).astype(bf16),
            }
        )

    res = run_bass_kernel_spmd(nc, in_maps, list(range(NCORES)))
    LAST_RESULT = res

    out = np.empty((B, S, D), dtype=np.float32)
    for c in range(NCORES):
        b, hg = divmod(c, 2)
        out[b, :, hg * DL : (hg + 1) * DL] = res.results[c]["outT"].T
    return out
